# revision 21
# baseline (speedup 1.0000x reference)
"""Trainium2 Bass kernel for nn_ClusterMemory (scatter_memory).

Reference computation (B=256, D=2048, S=65536, TEMP=0.05):
    x = inputs / ||inputs||_row            # [B, D]
    logits = (x @ features.T) / TEMP       # [B, S]
    loss = mean_i( logsumexp(logits[i,:]) - logits[i, targets[i]] )

Both x rows and feature rows are L2-normalized, so every logit is a
cosine / TEMP, bounded to [-20, 20] -> exp() never overflows in f32 and no
max-subtraction pass is needed.  Each of the 8 cores returns
S_shard[i] = sum_j exp(logits[i, j]) over its 8192-row shard of the memory
bank (features sharded row-wise).  The softmax normalizer combine (a
[256]-vector sum over 8 shards) and the target-logit term (256 dot
products) are done on host in f64.

Device kernel per core (streams its feature shard once), fp8 DoubleRow:
    xT      [128, 16, 256]           (normalized inputs, packed, *64)
    featsT  [N_CHUNKS, 128, 16, 512] (feature shard, packed, *64)
    s_out   [128, 2]  f32            (s_out[p, h] = S for item h*128+p)

    for each of 16 j-chunks (512 memory rows):
        DMA chunk -> SBUF [128, 16, 512]
        8 k-pair steps x 2 batch halves of DoubleRow matmuls -> PSUM
        ACT: exp(psum / TEMP) with accum_out -> per-row partial sum
    reduce the 16 partial sums per half, DMA [128, 2] out.

Perf notes (HW-traced):  steady-state MMs issue at the N=512 warm peak
(216 ns, LDWEIGHTS hidden), so the schedule is head/tail-bound:
  - xT DMA is split so k-pair 0 lands first and the first MM isn't gated
    on the full 0.5 MB transfer;
  - chunk 0 is split into 4 sub-DMAs on the sync HWDGE ring while xT and
    chunk 1 ride the scalar HWDGE ring concurrently (issue-side
    parallelism -- the two rings share the same 16 SDMA engines);
  - a burst of warm-up matmuls on a memset tile runs during the ~7 us
    framework preamble so the PE HAM clock-gate reaches 2.4 GHz before
    the first real matmul;
  - k-pair outer / batch-half inner order halves the DMA urgency per
    k-tile at the head of each chunk.
"""

import os
import numpy as np

import concourse.bacc as bacc
import concourse.bass as bass
import concourse.mybir as mybir
import concourse.tile as tile

B = 256
D = 2048
S = 65536
TEMP = 0.05
N_CORES = 8
SHARD = S // N_CORES          # 8192 rows of the memory bank per core
JC = 512                      # j-chunk width (one PSUM bank of f32)
N_CHUNKS = SHARD // JC        # 16
KT = D // 128                 # 16 k-tiles of 128

MODE = os.environ.get("CM_MODE", "fp8")
N_WARM = int(os.environ.get("CM_WARM", "14"))

# e4m3 normal range starts at 2^-6; x/feats components are ~N(0, 1/2048)
# (sigma 0.022), so scale by 2^6 to keep ~99% of them normal.  The matmul
# then computes (64x)*(64f); the 1/4096 is folded into the ACT exp scale.
FP8_SCALE = 64.0


def build_nc(mode=MODE):
    f32 = mybir.dt.float32
    if mode == "bf16":
        in_dt = mybir.dt.bfloat16
    elif mode == "fp8":
        in_dt = mybir.dt.float8e4
    elif mode == "f32r":
        in_dt = mybir.dt.float32r
    else:
        in_dt = f32
    act_scale = 1.0 / TEMP
    if mode == "fp8":
        act_scale /= FP8_SCALE * FP8_SCALE
    dr = mybir.MatmulPerfMode.DoubleRow

    nc = bacc.Bacc("TRN2", target_bir_lowering=False, debug=False,
                   num_devices=N_CORES)
    xT_d = nc.dram_tensor("xT", [128, KT, B], in_dt, kind="ExternalInput")
    featsT_d = nc.dram_tensor("featsT", [N_CHUNKS, 128, KT, JC], in_dt,
                              kind="ExternalInput")
    # per-chunk partial sums; the 16-way reduce happens on host.  128 B
    # per partition keeps the output DMA descriptors above the SDMA
    # read-modify-write threshold (a [128, 2] output writes 8 B lines).
    s_d = nc.dram_tensor("s_out", [128, 2, N_CHUNKS], f32,
                         kind="ExternalOutput")

    with tile.TileContext(nc) as tc:
        with (
            tc.tile_pool(name="xpool", bufs=1) as xpool,
            tc.tile_pool(name="fpool", bufs=6) as fpool,
            tc.tile_pool(name="spool", bufs=1) as spool,
            tc.tile_pool(name="jpool", bufs=4) as jpool,
            tc.tile_pool(name="warm", bufs=1) as wpool,
            tc.tile_pool(name="psum", bufs=6, space="PSUM") as ppool,
            tc.tile_pool(name="wps", bufs=1, space="PSUM") as wppool,
        ):
            # --- PE warm-up: keep the PE array busy through the HAM
            # activity window during the DMA head so real matmuls start
            # at 2.4 GHz.  Reads a memset tile, writes a junk PSUM bank.
            if mode == "fp8" and N_WARM:
                wmov = wpool.tile([128, 2, 256], in_dt)
                nc.gpsimd.memset(wmov[:], 0)
                wps = wppool.tile([128, 256], f32)
                for _ in range(N_WARM):
                    nc.tensor.matmul(wps[:], wmov[:, :, 0:128],
                                     wmov[:], start=True, stop=True,
                                     perf_mode=dr)

            # --- input DMAs.  The engines leave the Tile prologue
            # barrier staggered (Scalar first, GpSimd second, Sync
            # last), so the head-critical transfers are spread over
            # three issue engines: chunk 0 on the scalar HWDGE ring
            # (earliest issuer), xT on the GpSimd SWDGE path, and the
            # steady chunk stream on the sync HWDGE ring.  Chunks 1-2
            # ride sync in halves for finer completion granularity
            # during the ramp.
            xT = xpool.tile([128, KT, B], in_dt)
            nc.gpsimd.dma_start(out=xT[:, 0:2, :], in_=xT_d[:, 0:2, :])
            nc.gpsimd.dma_start(out=xT[:, 2:KT, :], in_=xT_d[:, 2:KT, :])

            f_tiles = {}
            f_tiles[0] = fpool.tile([128, KT, JC], in_dt, tag="feats",
                                    name="f_tile")
            for lo, hi in ((0, 2), (2, 4), (4, 8), (8, 16)):
                nc.scalar.dma_start(out=f_tiles[0][:, lo:hi, :],
                                    in_=featsT_d[0, :, lo:hi, :])
            for jc in (1, 2):
                f_tiles[jc] = fpool.tile([128, KT, JC], in_dt, tag="feats",
                                         name="f_tile")
                nc.sync.dma_start(out=f_tiles[jc][:, 0:8, :],
                                  in_=featsT_d[jc, :, 0:8, :])
                nc.sync.dma_start(out=f_tiles[jc][:, 8:16, :],
                                  in_=featsT_d[jc, :, 8:16, :])

            sums = spool.tile([128, 2, N_CHUNKS], f32)

            # Filler warm matmuls spread inside the first chunks' matmul
            # groups: during the DMA ramp the PE would stall there anyway,
            # and an idle window would drop the HAM clock back to 1.2 GHz.
            # {(jc, t): count} -> fillers after the (t, bh=1) matmul.
            fills = {(0, 3): 2, (0, 7): 2, (1, 3): 1, (1, 7): 1} \
                if (mode == "fp8" and N_WARM) else {}

            for jc in range(N_CHUNKS):
                if jc not in f_tiles:
                    f_tiles[jc] = fpool.tile([128, KT, JC], in_dt,
                                             tag="feats", name="f_tile")
                    nc.sync.dma_start(out=f_tiles[jc][:], in_=featsT_d[jc])
                f_tile = f_tiles[jc]
                ps = {bh: ppool.tile([128, JC], f32, tag="ps", name="ps")
                      for bh in range(2)}
                if mode == "fp8":
                    # DoubleRow: one matmul consumes two adjacent k-tiles;
                    # operands are [128, 2, dim] APs.  k-pair outer, batch
                    # half inner: each k-tile is needed at half the rate
                    # (eases the DMA ramp), and the two matmuls of a pair
                    # share the same moving AP, which keeps LDWEIGHTS
                    # fully hidden (batch-half-outer order measured ~43 ns
                    # slower per matmul).
                    for t in range(KT // 2):
                        for bh in range(2):
                            bsl = slice(bh * 128, (bh + 1) * 128)
                            nc.tensor.matmul(
                                ps[bh][:],
                                xT[:, 2 * t:2 * t + 2, bsl],
                                f_tile[:, 2 * t:2 * t + 2, :],
                                start=(t == 0), stop=(t == KT // 2 - 1),
                                perf_mode=dr)
                        for _ in range(fills.get((jc, t), 0)):
                            nc.tensor.matmul(wps[:], wmov[:, :, 0:128],
                                             wmov[:], start=True, stop=True,
                                             perf_mode=dr)
                else:
                    for kk in range(KT):
                        for bh in range(2):
                            bsl = slice(bh * 128, (bh + 1) * 128)
                            nc.tensor.matmul(
                                ps[bh][:], xT[:, kk, bsl], f_tile[:, kk, :],
                                start=(kk == 0), stop=(kk == KT - 1))
                for bh in range(2):
                    junk = jpool.tile([128, JC], f32, tag="junk")
                    nc.scalar.activation(
                        junk[:], ps[bh][:], mybir.ActivationFunctionType.Exp,
                        scale=act_scale,
                        accum_out=sums[:, bh, jc:jc + 1])

            nc.sync.dma_start(out=s_d[:], in_=sums[:])

    nc.compile()
    return nc


_NC_CACHE = {}


def _get_nc(mode=MODE):
    if mode not in _NC_CACHE:
        _NC_CACHE[mode] = build_nc(mode)
    return _NC_CACHE[mode]


def host_prep(inputs, features, mode=MODE):
    """Normalize/transpose/pack on host; returns (x_norm_f32, in_maps)."""
    x = np.asarray(inputs, dtype=np.float32)
    x = x / np.linalg.norm(x, axis=1, keepdims=True)
    scale = np.float32(1.0)
    if mode == "bf16":
        import ml_dtypes
        np_dt = ml_dtypes.bfloat16
    elif mode == "fp8":
        import ml_dtypes
        np_dt = ml_dtypes.float8_e4m3
        scale = np.float32(FP8_SCALE)
    else:
        np_dt = np.float32

    # xT[p, kk, b] = x[b, kk*128 + p]
    xT = np.ascontiguousarray(
        (x.T * scale).reshape(KT, 128, B).transpose(1, 0, 2).astype(np_dt))

    feats = np.asarray(features, dtype=np.float32)
    if mode == "fp8":
        feats = feats * scale
    in_maps = []
    for c in range(N_CORES):
        # shardT[k, j] = feats[c*SHARD + j, k]; packed[jc, p, kk, j] =
        # shardT[kk*128 + p, jc*JC + j]
        shardT = feats[c * SHARD:(c + 1) * SHARD].T       # [D, SHARD] view
        packed = np.ascontiguousarray(
            shardT.reshape(KT, 128, N_CHUNKS, JC).transpose(2, 1, 0, 3)
            .astype(np_dt))
        in_maps.append({"xT": xT, "featsT": packed})
    return x, in_maps


def combine(x, features, targets, core_outs):
    """Host combine: sum shard normalizers, add the target-logit term."""
    S_total = np.zeros(B, dtype=np.float64)
    for out in core_outs:
        s = out["s_out"].astype(np.float64).sum(axis=2)   # [128, 2]
        S_total += s.T.reshape(-1)                # item i = h*128 + p
    t = np.asarray(targets).astype(np.int64)
    f_t = np.asarray(features, dtype=np.float32)[t]          # [B, D]
    l_tgt = np.einsum("ij,ij->i", x.astype(np.float64),
                      f_t.astype(np.float64)) / TEMP
    loss = np.mean(np.log(S_total) - l_tgt)
    return np.array(loss, dtype=np.float32)


def kernel(**inputs):
    from concourse.bass_utils import run_bass_kernel_spmd

    x, in_maps = host_prep(inputs["inputs"], inputs["features"])
    nc = _get_nc()
    res = run_bass_kernel_spmd(nc, in_maps, list(range(N_CORES)))
    return combine(x, inputs["features"], inputs["targets"], res.results)


# revision 23
# speedup vs baseline: 1.0980x; 1.0980x over previous
"""Trainium2 Bass kernel for nn_ClusterMemory (scatter_memory).

Reference computation (B=256, D=2048, S=65536, TEMP=0.05):
    x = inputs / ||inputs||_row            # [B, D]
    logits = (x @ features.T) / TEMP       # [B, S]
    loss = mean_i( logsumexp(logits[i,:]) - logits[i, targets[i]] )

Both x rows and feature rows are L2-normalized, so every logit is a
cosine / TEMP, bounded to [-20, 20] -> exp() never overflows in f32 and no
max-subtraction pass is needed.  Each of the 8 cores returns
S_shard[i] = sum_j exp(logits[i, j]) over its 8192-row shard of the memory
bank (features sharded row-wise).  The softmax normalizer combine (a
[256]-vector sum over 8 shards) and the target-logit term (256 dot
products) are done on host in f64.

Device kernel per core (streams its feature shard once), fp8 DoubleRow:
    xT      [128, 16, 256]           (normalized inputs, packed, *64)
    featsT  [N_CHUNKS, 128, 16, 512] (feature shard, packed, *64)
    s_out   [128, 2]  f32            (s_out[p, h] = S for item h*128+p)

    for each of 16 j-chunks (512 memory rows):
        DMA chunk -> SBUF [128, 16, 512]
        8 k-pair steps x 2 batch halves of DoubleRow matmuls -> PSUM
        ACT: exp(psum / TEMP) with accum_out -> per-row partial sum
    reduce the 16 partial sums per half, DMA [128, 2] out.

Perf notes (HW-traced):  steady-state MMs issue at the N=512 warm peak
(216 ns, LDWEIGHTS hidden), so the schedule is head/tail-bound:
  - xT DMA is split so k-pair 0 lands first and the first MM isn't gated
    on the full 0.5 MB transfer;
  - chunk 0 is split into 4 sub-DMAs on the sync HWDGE ring while xT and
    chunk 1 ride the scalar HWDGE ring concurrently (issue-side
    parallelism -- the two rings share the same 16 SDMA engines);
  - a burst of warm-up matmuls on a memset tile runs during the ~7 us
    framework preamble so the PE HAM clock-gate reaches 2.4 GHz before
    the first real matmul;
  - k-pair outer / batch-half inner order halves the DMA urgency per
    k-tile at the head of each chunk.
"""

import os
import numpy as np

import concourse.bacc as bacc
import concourse.bass as bass
import concourse.mybir as mybir
import concourse.tile as tile

B = 256
D = 2048
S = 65536
TEMP = 0.05
N_CORES = 8
SHARD = S // N_CORES          # 8192 rows of the memory bank per core
JC = 512                      # j-chunk width (one PSUM bank of f32)
N_CHUNKS = SHARD // JC        # 16
KT = D // 128                 # 16 k-tiles of 128

MODE = os.environ.get("CM_MODE", "fp8")
N_WARM = int(os.environ.get("CM_WARM", "16"))

# e4m3 normal range starts at 2^-6; x/feats components are ~N(0, 1/2048)
# (sigma 0.022), so scale by 2^6 to keep ~99% of them normal.  The matmul
# then computes (64x)*(64f); the 1/4096 is folded into the ACT exp scale.
FP8_SCALE = 64.0


def build_nc(mode=MODE):
    f32 = mybir.dt.float32
    if mode == "bf16":
        in_dt = mybir.dt.bfloat16
    elif mode == "fp8":
        in_dt = mybir.dt.float8e4
    elif mode == "f32r":
        in_dt = mybir.dt.float32r
    else:
        in_dt = f32
    act_scale = 1.0 / TEMP
    if mode == "fp8":
        act_scale /= FP8_SCALE * FP8_SCALE
    dr = mybir.MatmulPerfMode.DoubleRow

    nc = bacc.Bacc("TRN2", target_bir_lowering=False, debug=False,
                   num_devices=N_CORES)
    xT_d = nc.dram_tensor("xT", [128, KT, B], in_dt, kind="ExternalInput")
    featsT_d = nc.dram_tensor("featsT", [N_CHUNKS, 128, KT, JC], in_dt,
                              kind="ExternalInput")
    # per-chunk partial sums; the 16-way reduce happens on host.  128 B
    # per partition keeps the output DMA descriptors above the SDMA
    # read-modify-write threshold (a [128, 2] output writes 8 B lines).
    s_d = nc.dram_tensor("s_out", [128, 2, N_CHUNKS], f32,
                         kind="ExternalOutput")

    with tile.TileContext(nc) as tc:
        with (
            tc.tile_pool(name="xpool", bufs=1) as xpool,
            tc.tile_pool(name="fpool", bufs=6) as fpool,
            tc.tile_pool(name="spool", bufs=1) as spool,
            tc.tile_pool(name="jpool", bufs=4) as jpool,
            tc.tile_pool(name="warm", bufs=1) as wpool,
            tc.tile_pool(name="psum", bufs=6, space="PSUM") as ppool,
            tc.tile_pool(name="wps", bufs=1, space="PSUM") as wppool,
        ):
            # --- PE warm-up: keep the PE array busy through the HAM
            # activity window during the DMA head so real matmuls start
            # at 2.4 GHz.  Reads a memset tile, writes a junk PSUM bank.
            if mode == "fp8" and N_WARM:
                wmov = wpool.tile([128, 2, 256], in_dt)
                nc.gpsimd.memset(wmov[:], 0)
                wps = wppool.tile([128, 256], f32)
                for _ in range(N_WARM):
                    nc.tensor.matmul(wps[:], wmov[:, :, 0:128],
                                     wmov[:], start=True, stop=True,
                                     perf_mode=dr)

            # --- input DMAs.  All feature chunks stream on the sync
            # HWDGE ring in consumption order (parallel queues drain at
            # equal packet rates, which destroys need-ordering); only
            # xT rides the scalar ring, split so k-pair 0 lands first.
            # Chunk 0 is sub-split and chunks 1-2 ride in halves for
            # finer completion granularity during the ramp.
            xT = xpool.tile([128, KT, B], in_dt)
            nc.scalar.dma_start(out=xT[:, 0:2, :], in_=xT_d[:, 0:2, :])
            nc.scalar.dma_start(out=xT[:, 2:KT, :], in_=xT_d[:, 2:KT, :])

            f_tiles = {}
            f_tiles[0] = fpool.tile([128, KT, JC], in_dt, tag="feats",
                                    name="f_tile")
            for lo, hi in ((0, 2), (2, 4), (4, 8), (8, 16)):
                nc.sync.dma_start(out=f_tiles[0][:, lo:hi, :],
                                  in_=featsT_d[0, :, lo:hi, :])
            for jc in (1, 2):
                f_tiles[jc] = fpool.tile([128, KT, JC], in_dt, tag="feats",
                                         name="f_tile")
                nc.sync.dma_start(out=f_tiles[jc][:, 0:8, :],
                                  in_=featsT_d[jc, :, 0:8, :])
                nc.sync.dma_start(out=f_tiles[jc][:, 8:16, :],
                                  in_=featsT_d[jc, :, 8:16, :])

            sums = spool.tile([128, 2, N_CHUNKS], f32)

            # Filler warm matmuls spread inside the first chunks' matmul
            # groups: during the DMA ramp the PE would stall there anyway,
            # and an idle window would drop the HAM clock back to 1.2 GHz.
            # {(jc, t): count} -> fillers after the (t, bh=1) matmul.
            fills = {(0, 3): 2, (0, 7): 2, (1, 3): 1, (1, 7): 1} \
                if (mode == "fp8" and N_WARM) else {}

            for jc in range(N_CHUNKS):
                if jc not in f_tiles:
                    f_tiles[jc] = fpool.tile([128, KT, JC], in_dt,
                                             tag="feats", name="f_tile")
                    nc.sync.dma_start(out=f_tiles[jc][:], in_=featsT_d[jc])
                f_tile = f_tiles[jc]
                ps = {bh: ppool.tile([128, JC], f32, tag="ps", name="ps")
                      for bh in range(2)}
                if mode == "fp8":
                    # DoubleRow: one matmul consumes two adjacent k-tiles;
                    # operands are [128, 2, dim] APs.  k-pair outer, batch
                    # half inner: each k-tile is needed at half the rate
                    # (eases the DMA ramp), and the two matmuls of a pair
                    # share the same moving AP, which keeps LDWEIGHTS
                    # fully hidden (batch-half-outer order measured ~43 ns
                    # slower per matmul).
                    for t in range(KT // 2):
                        for bh in range(2):
                            bsl = slice(bh * 128, (bh + 1) * 128)
                            nc.tensor.matmul(
                                ps[bh][:],
                                xT[:, 2 * t:2 * t + 2, bsl],
                                f_tile[:, 2 * t:2 * t + 2, :],
                                start=(t == 0), stop=(t == KT // 2 - 1),
                                perf_mode=dr)
                        for _ in range(fills.get((jc, t), 0)):
                            nc.tensor.matmul(wps[:], wmov[:, :, 0:128],
                                             wmov[:], start=True, stop=True,
                                             perf_mode=dr)
                else:
                    for kk in range(KT):
                        for bh in range(2):
                            bsl = slice(bh * 128, (bh + 1) * 128)
                            nc.tensor.matmul(
                                ps[bh][:], xT[:, kk, bsl], f_tile[:, kk, :],
                                start=(kk == 0), stop=(kk == KT - 1))
                for bh in range(2):
                    junk = jpool.tile([128, JC], f32, tag="junk")
                    nc.scalar.activation(
                        junk[:], ps[bh][:], mybir.ActivationFunctionType.Exp,
                        scale=act_scale,
                        accum_out=sums[:, bh, jc:jc + 1])

            nc.sync.dma_start(out=s_d[:], in_=sums[:])

    nc.compile()
    return nc


_NC_CACHE = {}


def _get_nc(mode=MODE):
    if mode not in _NC_CACHE:
        _NC_CACHE[mode] = build_nc(mode)
    return _NC_CACHE[mode]


def host_prep(inputs, features, mode=MODE):
    """Normalize/transpose/pack on host; returns (x_norm_f32, in_maps)."""
    x = np.asarray(inputs, dtype=np.float32)
    x = x / np.linalg.norm(x, axis=1, keepdims=True)
    scale = np.float32(1.0)
    if mode == "bf16":
        import ml_dtypes
        np_dt = ml_dtypes.bfloat16
    elif mode == "fp8":
        import ml_dtypes
        np_dt = ml_dtypes.float8_e4m3
        scale = np.float32(FP8_SCALE)
    else:
        np_dt = np.float32

    # xT[p, kk, b] = x[b, kk*128 + p]
    xT = np.ascontiguousarray(
        (x.T * scale).reshape(KT, 128, B).transpose(1, 0, 2).astype(np_dt))

    feats = np.asarray(features, dtype=np.float32)
    if mode == "fp8":
        feats = feats * scale
    in_maps = []
    for c in range(N_CORES):
        # shardT[k, j] = feats[c*SHARD + j, k]; packed[jc, p, kk, j] =
        # shardT[kk*128 + p, jc*JC + j]
        shardT = feats[c * SHARD:(c + 1) * SHARD].T       # [D, SHARD] view
        packed = np.ascontiguousarray(
            shardT.reshape(KT, 128, N_CHUNKS, JC).transpose(2, 1, 0, 3)
            .astype(np_dt))
        in_maps.append({"xT": xT, "featsT": packed})
    return x, in_maps


def combine(x, features, targets, core_outs):
    """Host combine: sum shard normalizers, add the target-logit term."""
    S_total = np.zeros(B, dtype=np.float64)
    for out in core_outs:
        s = out["s_out"].astype(np.float64).sum(axis=2)   # [128, 2]
        S_total += s.T.reshape(-1)                # item i = h*128 + p
    t = np.asarray(targets).astype(np.int64)
    f_t = np.asarray(features, dtype=np.float32)[t]          # [B, D]
    l_tgt = np.einsum("ij,ij->i", x.astype(np.float64),
                      f_t.astype(np.float64)) / TEMP
    loss = np.mean(np.log(S_total) - l_tgt)
    return np.array(loss, dtype=np.float32)


def kernel(**inputs):
    from concourse.bass_utils import run_bass_kernel_spmd

    x, in_maps = host_prep(inputs["inputs"], inputs["features"])
    nc = _get_nc()
    res = run_bass_kernel_spmd(nc, in_maps, list(range(N_CORES)))
    return combine(x, inputs["features"], inputs["targets"], res.results)
